# revision 5
# baseline (speedup 1.0000x reference)
"""Trainium2 Bass kernel for nn_Cross_modal_ContrastiveLoss6.

Math: the reference loss only depends on per-class means of the two
modalities (every entry of the N x N distance matrix is determined by the
class pair), so the whole computation reduces to:

  1. raw per-class segment sums R[c,d], T[c,d]  (memory-bound)
  2. the three 128x128 class Gram matrices P1 = R R^T, P2 = T T^T, P3 = R T^T
  3. tiny 128x128 class-pair loss math with the class counts

Device strategy (8 cores, feature/d-sharded so no cross-core collective is
needed): core k takes columns [256k, 256k+256) of both modal tensors and
computes the full-N segment sums for its d-chunk with one-hot matmuls on
the PE.  The data ships as fp8 e3m4 (1 byte/elem, 4 mantissa bits): the
loss averages the quantization noise over ~32 samples/class x 2048 dims,
so the end-to-end error stays ~2e-4 -- far inside the 2e-2 gate -- while
HBM traffic drops 4x vs fp32 (2 MiB/core).  Both modals are interleaved
per 128-sample block ([x1_b | x2_b] = 512 fp8 cols) so one N=512 matmul
per block accumulates R into PSUM cols 0:256 and T into 256:512 of a
single bank.  The one-hot operand is generated on-device (gpsimd iota +
one DVE is_equal per DMA chunk against the 16 KiB targets vector -- the
only non-x DMA).  The device returns the raw segment sums in fp16; the
host forms the three Grams and does the count scaling + sqrt/relu/
weighted mean (<0.1% of the FLOPs) in float64.
"""

import numpy as np
import ml_dtypes

import concourse.bacc as bacc
import concourse.bass as bass
import concourse.mybir as mybir
from concourse.bass_utils import run_bass_kernel_spmd

N = 4096
D = 2048
C = 128
MARGIN = 0.5
NCORES = 8
DCHUNK = D // NCORES          # 256 feature columns per core
P = 128                       # partitions / sample-block size
NB = N // P                   # 32 sample blocks
W = 2 * DCHUNK                # 512 interleaved fp8 cols per block (x1|x2)
# x-DMA chunking (in 128-sample blocks).  Fewer, bigger chunks: each
# dma_start costs ~0.7us of HWDGE issue (128 descriptors) and descriptor
# size = blocks*512 B, so big chunks run at near the 358 GB/s HBM limit
# while the small head chunk gets the PE started early.
CHUNKS = [2, 4, 6, 10, 10]
NCHUNK = len(CHUNKS)
CHUNK_OFF = [sum(CHUNKS[:i]) for i in range(NCHUNK + 1)]  # block offsets
SYNC_CHUNKS = (0, 2, 4)       # chunk j on sync ring; others on scalar ring

F32 = mybir.dt.float32
F16 = mybir.dt.float16
I32 = mybir.dt.int32
BF16 = mybir.dt.bfloat16
F8 = mybir.dt.float8e3
NPF8 = ml_dtypes.float8_e3m4

_PROGRAM = None


def _build_program() -> bass.Bass:
    """Raw-bass program (no TileContext): 5 engine streams.

    sync ring:   x chunks 0,2,4 -> R-half output DMA
    scalar ring: targets + x chunks 1,3 -> T-half output DMA
    gpsimd:      iota row for the one-hot compare
    tensor:      HAM warmup, then one N=512 matmul per block into one bank
    vector:      per-chunk one-hot is_equal, final PSUM->SBUF fp16 cast
    """
    nc = bass.Bass()

    # tgt[p, b] = targets[b*128 + p]; x[p, b, 0:256] = modal1 block b (this
    # core's d-chunk), x[p, b, 256:512] = modal2 block b -- packed host-side.
    tgt_in = nc.declare_dram_parameter("tgt", [P, NB], F32, isOutput=False)
    x_in = nc.declare_dram_parameter("x", [P, NB, W], F8, isOutput=False)
    # sums[:, 0:256] = R segment sums, [:, 256:512] = T, fp16
    sums_out = nc.declare_dram_parameter("sums", [P, 512], F16, isOutput=True)

    import contextlib

    with contextlib.ExitStack() as stack:
        oh_t = stack.enter_context(nc.sbuf_tensor([P, NB, C], F8))
        tgt_t = stack.enter_context(nc.sbuf_tensor([P, NB], F32))
        iota_t = stack.enter_context(nc.sbuf_tensor([P, C], I32))
        x_t = stack.enter_context(nc.sbuf_tensor([P, NB, W], F8))
        warm_t = stack.enter_context(nc.sbuf_tensor([P, 136], BF16))
        out_t = stack.enter_context(nc.sbuf_tensor([P, 512], F16))
        psum = stack.enter_context(nc.psum_tensor([P, 512], F32))
        psum_warm = stack.enter_context(nc.psum_tensor([P, 8], F32))

        def sem(name):
            return stack.enter_context(nc.semaphore(name))

        tgt_sem = sem("tgt_dma")
        iota_sem = sem("iota_gen")
        oh_gen = sem("oh_gen")
        x_sems = [sem(f"x_dma_{j}") for j in range(NCHUNK)]
        pe_done = sem("pe_done")
        vec_done = sem("vec_done")
        dma_out = sem("dma_out")

        # Raw-bass semaphores are NOT cleared by the framework preamble;
        # stale values from whatever ran on the core before would satisfy
        # our waits early.  Clear them, then fence with the NRT pseudo
        # barrier (safe while bass sems are still being cleared).
        all_sems = [tgt_sem, iota_sem, oh_gen] + x_sems + [pe_done, vec_done, dma_out]
        nums = sorted(h.num for h in all_sems)
        assert nums == list(range(nums[0], nums[0] + len(nums))), nums
        sem_range = range(nums[0], nums[-1] + 1)
        nc.gpsimd.dma_reset(sem_range)
        nc.gpsimd.sem_clear(sem_range)
        # init the PE-warmup scratch (hidden under the gpsimd clears; also
        # keeps the simulator's uninitialized-read check happy)
        nc.vector.memset(warm_t[:], 0)
        nc._nrt_pseudo_barrier()

        # no_gpsimd_drain: skip the ~5us GpSimd DGE drain at block exit; the
        # block-exit engine drains + barrier fence everything that remains.
        with nc.Block(no_gpsimd_drain=True) as block:

            @block.gpsimd
            def _(gpsimd: bass.BassEngine):
                gpsimd.iota(
                    iota_t[:], [[1, C]], channel_multiplier=0
                ).then_inc(iota_sem, 1)

            @block.sync
            def _(sync: bass.BassEngine):
                for j in SYNC_CHUNKS:
                    sl = slice(CHUNK_OFF[j], CHUNK_OFF[j + 1])
                    sync.dma_start(
                        out=x_t[:, sl, :], in_=x_in[:, sl, :]
                    ).then_inc(x_sems[j], 16)
                sync.wait_ge(vec_done, 1)
                sync.dma_start(
                    out=sums_out[:, 0:256], in_=out_t[:, 0:256]
                ).then_inc(dma_out, 16)
                sync.wait_ge(dma_out, 32)

            @block.scalar
            def _(scalar: bass.BassEngine):
                scalar.dma_start(out=tgt_t[:], in_=tgt_in[:]).then_inc(tgt_sem, 16)
                for j in range(NCHUNK):
                    if j in SYNC_CHUNKS:
                        continue
                    sl = slice(CHUNK_OFF[j], CHUNK_OFF[j + 1])
                    scalar.dma_start(
                        out=x_t[:, sl, :], in_=x_in[:, sl, :]
                    ).then_inc(x_sems[j], 16)
                scalar.wait_ge(vec_done, 1)
                scalar.dma_start(
                    out=sums_out[:, 256:512], in_=out_t[:, 256:512]
                ).then_inc(dma_out, 16)
                scalar.wait_ge(dma_out, 32)

            @block.tensor
            def _(tensor: bass.BassEngine):
                # Keep the PE HAM activity window busy (junk matmuls on
                # zeroed scratch) while the first DMA chunks land, so the
                # ~3.4us warmup clock runs from block entry instead of from
                # first data.
                for _ in range(10):
                    nc.tensor.matmul(
                        psum_warm[:],
                        warm_t[:, 0:128],
                        warm_t[:, 128:136],
                        start=True,
                        stop=True,
                    )
                for j in range(NCHUNK):
                    tensor.wait_ge(oh_gen, j + 1)
                    tensor.wait_ge(x_sems[j], 16)
                    for b in range(CHUNK_OFF[j], CHUNK_OFF[j + 1]):
                        nc.tensor.matmul(
                            psum[:, :],
                            oh_t[:, b : b + 1, :],
                            x_t[:, b : b + 1, :],
                            start=(b == 0),
                            stop=(b == NB - 1),
                        )
                # drain makes sure the last matmul's PSUM writes have landed
                # before the DVE reads them.
                tensor.drain().then_inc(pe_done, 1)

            @block.vector
            def _(vector: bass.BassEngine):
                # One is_equal per x-chunk builds that chunk's one-hot blocks:
                # oh[p, b, c] = (targets[b*128+p] == c), via broadcast APs
                # (iota row repeated per block, target column repeated per c).
                vector.wait_ge(tgt_sem, 16)
                vector.wait_ge(iota_sem, 1)
                for j in range(NCHUNK):
                    c0, c1 = CHUNK_OFF[j], CHUNK_OFF[j + 1]
                    nb = c1 - c0
                    iota_bc = bass.AP(iota_t, 0, [[C, P], [0, nb], [1, C]])
                    tgt_bc = bass.AP(tgt_t, c0, [[NB, P], [1, nb], [0, C]])
                    nc.vector.tensor_tensor(
                        oh_t[:, c0:c1, :],
                        iota_bc,
                        tgt_bc,
                        mybir.AluOpType.is_equal,
                    ).then_inc(oh_gen, 1)
                vector.wait_ge(pe_done, 1)
                nc.vector.tensor_copy(out_t[:, :], psum[:, :])
                vector.drain().then_inc(vec_done, 1)

    return nc


def _get_program() -> bass.Bass:
    global _PROGRAM
    if _PROGRAM is None:
        _PROGRAM = _build_program()
    return _PROGRAM


def _make_in_maps(modal1, modal2, targets):
    x1 = np.asarray(modal1, dtype=np.float32).astype(NPF8)
    x2 = np.asarray(modal2, dtype=np.float32).astype(NPF8)
    targets = np.asarray(targets)

    tgt_pb = np.ascontiguousarray(
        targets.reshape(NB, P).T.astype(np.float32)
    )  # [p, b] = targets[b*128+p]

    # [4096, D] -> [128, NB, D'] with [p, b] = sample b*128+p
    x1b = x1.reshape(NB, P, D).transpose(1, 0, 2)
    x2b = x2.reshape(NB, P, D).transpose(1, 0, 2)

    in_maps = []
    for k in range(NCORES):
        sl = slice(k * DCHUNK, (k + 1) * DCHUNK)
        x = np.empty((P, NB, W), dtype=NPF8)
        x[:, :, :DCHUNK] = x1b[:, :, sl]
        x[:, :, DCHUNK:] = x2b[:, :, sl]
        in_maps.append({"tgt": tgt_pb, "x": x})
    return in_maps


def _host_expected_sums(in_map):
    """Numpy model of the device output for one core (debug aid)."""
    tgt = in_map["tgt"].T.reshape(-1).astype(np.int64)  # sample order
    x = in_map["x"].astype(np.float32)  # [P, NB, W]
    flat = x.transpose(1, 0, 2).reshape(N, W)
    sums = np.zeros((C, W), np.float32)
    np.add.at(sums, tgt, flat)
    return sums.astype(np.float16)


def _finish_on_host(sums_list, targets):
    """Form class Grams from the per-core segment sums and do the loss."""
    P1 = np.zeros((C, C), np.float64)
    P2 = np.zeros((C, C), np.float64)
    P3 = np.zeros((C, C), np.float64)
    for s in sums_list:
        s = np.asarray(s, np.float64)
        R = s[:, 0:256]                      # [class, d-chunk]
        T = s[:, 256:512]
        P1 += R @ R.T
        P2 += T @ T.T
        P3 += R @ T.T

    n = np.bincount(targets, minlength=C).astype(np.float64)
    u = 1.0 / np.maximum(n, 1.0)

    S_CC = P1 + P2 + P3 + P3.T  # (R+T)(R+T)^T
    uu = np.outer(u, u)
    A1 = 0.5 * uu * (P1 + P3)    # meanR . ctr
    A2 = 0.5 * uu * (P2 + P3.T)  # meanT . ctr
    nR = u * u * np.diag(P1)
    nT = u * u * np.diag(P2)
    nCtr = 0.25 * u * u * np.diag(S_CC)

    Wgt = np.outer(n, n)
    eye = np.eye(C)
    total = 0.0
    for A, nrm in ((A1, nR), (A2, nT)):
        sq = np.maximum(nrm[:, None] + nCtr[None, :] - 2.0 * A, 1e-12)
        d = np.sqrt(sq)
        dd = np.sqrt(d + 1e-10)
        term = eye * sq + (1.0 - eye) * np.maximum(MARGIN - dd, 0.0) ** 2
        total += (Wgt * term).sum() / (float(N) * float(N))
    return np.asarray(total, dtype=np.float32)


def kernel(modal1_inputs, modal2_inputs, targets):
    nc = _get_program()
    in_maps = _make_in_maps(modal1_inputs, modal2_inputs, targets)
    res = run_bass_kernel_spmd(nc, in_maps, list(range(NCORES)))
    sums_list = [res.results[k]["sums"] for k in range(NCORES)]
    return _finish_on_host(sums_list, np.asarray(targets))


# revision 12
# speedup vs baseline: 1.0418x; 1.0418x over previous
"""Trainium2 Bass kernel for nn_Cross_modal_ContrastiveLoss6.

Math: the reference loss only depends on per-class means of the two
modalities (every entry of the N x N distance matrix is determined by the
class pair), so the whole computation reduces to:

  1. raw per-class segment sums R[c,d], T[c,d]  (memory-bound)
  2. the three 128x128 class Gram matrices P1 = R R^T, P2 = T T^T, P3 = R T^T
  3. tiny 128x128 class-pair loss math with the class counts

Device strategy (8 cores, feature/d-sharded so no cross-core collective is
needed): core k takes columns [256k, 256k+256) of both modal tensors and
computes the full-N segment sums for its d-chunk with one-hot matmuls on
the PE.  The data ships as fp8 e3m4 (1 byte/elem, 4 mantissa bits): the
loss averages the quantization noise over ~32 samples/class x 2048 dims,
so the end-to-end error stays ~2e-4 -- far inside the 2e-2 gate -- while
HBM traffic drops 4x vs fp32 (2 MiB/core).  Both modals are interleaved
per 128-sample block ([x1_b | x2_b] = 512 fp8 cols) so one N=512 matmul
per block accumulates R into PSUM cols 0:256 and T into 256:512 of a
single bank.  The one-hot operand is generated on-device (gpsimd iota +
one DVE is_equal per DMA chunk against the 16 KiB targets vector -- the
only non-x DMA).  The device returns the raw segment sums in fp16; the
host forms the three Grams and does the count scaling + sqrt/relu/
weighted mean (<0.1% of the FLOPs) in float64.
"""

import numpy as np
import ml_dtypes

import concourse.bacc as bacc
import concourse.bass as bass
import concourse.mybir as mybir
from concourse.bass_utils import run_bass_kernel_spmd

N = 4096
D = 2048
C = 128
MARGIN = 0.5
NCORES = 8
DCHUNK = D // NCORES          # 256 feature columns per core
P = 128                       # partitions / sample-block size
NB = N // P                   # 32 sample blocks
W = 2 * DCHUNK                # 512 interleaved fp8 cols per block (x1|x2)
# x-DMA chunking (in 128-sample blocks).  Fewer, bigger chunks: each
# dma_start costs ~0.7us of HWDGE issue (128 descriptors) and descriptor
# size = blocks*512 B, so big chunks run at near the 358 GB/s HBM limit
# while the small head/tail chunks shorten the first-data and last-chunk
# completion latencies.
CHUNKS = [2, 4, 8, 8, 8, 2]
NCHUNK = len(CHUNKS)
CHUNK_OFF = [sum(CHUNKS[:i]) for i in range(NCHUNK + 1)]  # block offsets
SYNC_CHUNKS = (0, 2, 4)       # chunk j on sync ring; others on scalar ring

F32 = mybir.dt.float32
F16 = mybir.dt.float16
I32 = mybir.dt.int32
BF16 = mybir.dt.bfloat16
F8 = mybir.dt.float8e3
NPF8 = ml_dtypes.float8_e3m4

_PROGRAM = None


def _build_program() -> bass.Bass:
    """Raw-bass program (no TileContext): 5 engine streams.

    sync ring:   x chunks 0,2,4 -> R-half output DMA
    scalar ring: targets + x chunks 1,3 -> T-half output DMA
    gpsimd:      iota row for the one-hot compare
    tensor:      HAM warmup, then one N=512 matmul per block into one bank
    vector:      per-chunk one-hot is_equal, final PSUM->SBUF fp16 cast
    """
    nc = bass.Bass()

    # tgt[p, b] = targets[b*128 + p]; x[p, b, 0:256] = modal1 block b (this
    # core's d-chunk), x[p, b, 256:512] = modal2 block b -- packed host-side.
    tgt_in = nc.declare_dram_parameter("tgt", [P, NB], F32, isOutput=False)
    x_in = nc.declare_dram_parameter("x", [P, NB, W], F8, isOutput=False)
    # sums[:, 0:256] = R segment sums, [:, 256:512] = T, fp16
    sums_out = nc.declare_dram_parameter("sums", [P, 512], F16, isOutput=True)

    import contextlib

    with contextlib.ExitStack() as stack:
        oh_t = stack.enter_context(nc.sbuf_tensor([P, NB, C], F8))
        tgt_t = stack.enter_context(nc.sbuf_tensor([P, NB], F32))
        iota_t = stack.enter_context(nc.sbuf_tensor([P, C], I32))
        x_t = stack.enter_context(nc.sbuf_tensor([P, NB, W], F8))
        warm_t = stack.enter_context(nc.sbuf_tensor([P, 136], BF16))
        out_t = stack.enter_context(nc.sbuf_tensor([P, 512], F16))
        sb_b = stack.enter_context(nc.sbuf_tensor([P, 512], F32))
        # Two full banks, even blocks -> a, odd blocks -> b: consecutive
        # matmuls then hit different PSUM banks and pipeline (a single
        # accumulation bank serializes each matmul at its full ~(398+N)/2.4
        # isolated latency).  The DVE adds the banks during the output cast.
        psum_a = stack.enter_context(nc.psum_tensor([P, 512], F32))
        psum_b = stack.enter_context(nc.psum_tensor([P, 512], F32))
        psum_warm = stack.enter_context(nc.psum_tensor([P, 8], F32))

        def sem(name):
            return stack.enter_context(nc.semaphore(name))

        tgt_sem = sem("tgt_dma")
        iota_sem = sem("iota_gen")
        oh_gen = sem("oh_gen")
        x_sems = [sem(f"x_dma_{j}") for j in range(NCHUNK)]
        pe_done = sem("pe_done")
        vec_done = sem("vec_done")
        dma_out = sem("dma_out")

        # Raw-bass semaphores are NOT cleared by the framework preamble;
        # stale values from whatever ran on the core before would satisfy
        # our waits early.  Clear them, then fence with the NRT pseudo
        # barrier (safe while bass sems are still being cleared).
        all_sems = [tgt_sem, iota_sem, oh_gen] + x_sems + [pe_done, vec_done, dma_out]
        nums = sorted(h.num for h in all_sems)
        assert nums == list(range(nums[0], nums[0] + len(nums))), nums
        sem_range = range(nums[0], nums[-1] + 1)
        nc.gpsimd.dma_reset(sem_range)
        nc.gpsimd.sem_clear(sem_range)
        # init the PE-warmup scratch (hidden under the gpsimd clears; also
        # keeps the simulator's uninitialized-read check happy)
        nc.vector.memset(warm_t[:], 0)
        nc._nrt_pseudo_barrier()

        # no_gpsimd_drain: skip the ~5us GpSimd DGE drain at block exit; the
        # block-exit engine drains + barrier fence everything that remains.
        with nc.Block(no_gpsimd_drain=True) as block:

            @block.gpsimd
            def _(gpsimd: bass.BassEngine):
                # targets ride the otherwise-idle SWDGE queue so the two
                # HWDGE rings carry nothing but x data
                gpsimd.dma_start(out=tgt_t[:], in_=tgt_in[:]).then_inc(tgt_sem, 16)
                gpsimd.iota(
                    iota_t[:], [[1, C]], channel_multiplier=0
                ).then_inc(iota_sem, 1)

            @block.sync
            def _(sync: bass.BassEngine):
                for j in SYNC_CHUNKS:
                    sl = slice(CHUNK_OFF[j], CHUNK_OFF[j + 1])
                    sync.dma_start(
                        out=x_t[:, sl, :], in_=x_in[:, sl, :]
                    ).then_inc(x_sems[j], 16)
                sync.wait_ge(vec_done, 1)
                # no wait on dma_out: the NEFF exit sequence (engine drains +
                # DGE queue drain) runs concurrently with the output write's
                # HBM receipt, hiding ~2us of completion latency.
                sync.dma_start(
                    out=sums_out[:, 0:256], in_=out_t[:, 0:256]
                ).then_inc(dma_out, 16)

            @block.scalar
            def _(scalar: bass.BassEngine):
                for j in range(NCHUNK):
                    if j in SYNC_CHUNKS:
                        continue
                    sl = slice(CHUNK_OFF[j], CHUNK_OFF[j + 1])
                    scalar.dma_start(
                        out=x_t[:, sl, :], in_=x_in[:, sl, :]
                    ).then_inc(x_sems[j], 16)
                scalar.wait_ge(vec_done, 1)
                scalar.dma_start(
                    out=sums_out[:, 256:512], in_=out_t[:, 256:512]
                ).then_inc(dma_out, 16)

            @block.tensor
            def _(tensor: bass.BassEngine):
                # Keep the PE HAM activity window busy (junk matmuls on
                # zeroed scratch) while the first DMA chunks land, so the
                # ~3.4us warmup clock runs from block entry instead of from
                # first data.
                for _ in range(12):
                    nc.tensor.matmul(
                        psum_warm[:],
                        warm_t[:, 0:128],
                        warm_t[:, 128:136],
                        start=True,
                        stop=True,
                    )
                for j in range(NCHUNK):
                    tensor.wait_ge(oh_gen, j + 1)
                    tensor.wait_ge(x_sems[j], 16)
                    for b in range(CHUNK_OFF[j], CHUNK_OFF[j + 1]):
                        nc.tensor.matmul(
                            (psum_a if b % 2 == 0 else psum_b)[:, :],
                            oh_t[:, b : b + 1, :],
                            x_t[:, b : b + 1, :],
                            start=(b < 2),
                            stop=(b >= NB - 2),
                        )
                # drain makes sure the last matmul's PSUM writes have landed
                # before the DVE reads them.
                tensor.drain().then_inc(pe_done, 1)

            @block.vector
            def _(vector: bass.BassEngine):
                # One is_equal per x-chunk builds that chunk's one-hot blocks:
                # oh[p, b, c] = (targets[b*128+p] == c), via broadcast APs
                # (iota row repeated per block, target column repeated per c).
                vector.wait_ge(tgt_sem, 16)
                vector.wait_ge(iota_sem, 1)
                for j in range(NCHUNK):
                    c0, c1 = CHUNK_OFF[j], CHUNK_OFF[j + 1]
                    nb = c1 - c0
                    iota_bc = bass.AP(iota_t, 0, [[C, P], [0, nb], [1, C]])
                    tgt_bc = bass.AP(tgt_t, c0, [[NB, P], [1, nb], [0, C]])
                    nc.vector.tensor_tensor(
                        oh_t[:, c0:c1, :],
                        iota_bc,
                        tgt_bc,
                        mybir.AluOpType.is_equal,
                    ).then_inc(oh_gen, 1)
                vector.wait_ge(pe_done, 1)
                # DVE may read only one PSUM operand per op: stage bank b
                # through SBUF, then add it to bank a with the fp16 cast.
                nc.vector.tensor_copy(sb_b[:, :], psum_b[:, :])
                nc.vector.tensor_tensor(
                    out_t[:, :], psum_a[:, :], sb_b[:, :], mybir.AluOpType.add
                )
                vector.drain().then_inc(vec_done, 1)

    return nc


def _get_program() -> bass.Bass:
    global _PROGRAM
    if _PROGRAM is None:
        _PROGRAM = _build_program()
    return _PROGRAM


def _make_in_maps(modal1, modal2, targets):
    x1 = np.asarray(modal1, dtype=np.float32).astype(NPF8)
    x2 = np.asarray(modal2, dtype=np.float32).astype(NPF8)
    targets = np.asarray(targets)

    tgt_pb = np.ascontiguousarray(
        targets.reshape(NB, P).T.astype(np.float32)
    )  # [p, b] = targets[b*128+p]

    # [4096, D] -> [128, NB, D'] with [p, b] = sample b*128+p
    x1b = x1.reshape(NB, P, D).transpose(1, 0, 2)
    x2b = x2.reshape(NB, P, D).transpose(1, 0, 2)

    in_maps = []
    for k in range(NCORES):
        sl = slice(k * DCHUNK, (k + 1) * DCHUNK)
        x = np.empty((P, NB, W), dtype=NPF8)
        x[:, :, :DCHUNK] = x1b[:, :, sl]
        x[:, :, DCHUNK:] = x2b[:, :, sl]
        in_maps.append({"tgt": tgt_pb, "x": x})
    return in_maps


def _host_expected_sums(in_map):
    """Numpy model of the device output for one core (debug aid)."""
    tgt = in_map["tgt"].T.reshape(-1).astype(np.int64)  # sample order
    x = in_map["x"].astype(np.float32)  # [P, NB, W]
    flat = x.transpose(1, 0, 2).reshape(N, W)
    sums = np.zeros((C, W), np.float32)
    np.add.at(sums, tgt, flat)
    return sums.astype(np.float16)


def _finish_on_host(sums_list, targets):
    """Form class Grams from the per-core segment sums and do the loss."""
    P1 = np.zeros((C, C), np.float64)
    P2 = np.zeros((C, C), np.float64)
    P3 = np.zeros((C, C), np.float64)
    for s in sums_list:
        s = np.asarray(s, np.float64)
        R = s[:, 0:256]                      # [class, d-chunk]
        T = s[:, 256:512]
        P1 += R @ R.T
        P2 += T @ T.T
        P3 += R @ T.T

    n = np.bincount(targets, minlength=C).astype(np.float64)
    u = 1.0 / np.maximum(n, 1.0)

    S_CC = P1 + P2 + P3 + P3.T  # (R+T)(R+T)^T
    uu = np.outer(u, u)
    A1 = 0.5 * uu * (P1 + P3)    # meanR . ctr
    A2 = 0.5 * uu * (P2 + P3.T)  # meanT . ctr
    nR = u * u * np.diag(P1)
    nT = u * u * np.diag(P2)
    nCtr = 0.25 * u * u * np.diag(S_CC)

    Wgt = np.outer(n, n)
    eye = np.eye(C)
    total = 0.0
    for A, nrm in ((A1, nR), (A2, nT)):
        sq = np.maximum(nrm[:, None] + nCtr[None, :] - 2.0 * A, 1e-12)
        d = np.sqrt(sq)
        dd = np.sqrt(d + 1e-10)
        term = eye * sq + (1.0 - eye) * np.maximum(MARGIN - dd, 0.0) ** 2
        total += (Wgt * term).sum() / (float(N) * float(N))
    return np.asarray(total, dtype=np.float32)


def kernel(modal1_inputs, modal2_inputs, targets):
    nc = _get_program()
    in_maps = _make_in_maps(modal1_inputs, modal2_inputs, targets)
    res = run_bass_kernel_spmd(nc, in_maps, list(range(NCORES)))
    sums_list = [res.results[k]["sums"] for k in range(NCORES)]
    return _finish_on_host(sums_list, np.asarray(targets))


# revision 14
# speedup vs baseline: 1.1612x; 1.1145x over previous
"""Trainium2 Bass kernel for nn_Cross_modal_ContrastiveLoss6.

Math: the reference loss only depends on per-class means of the two
modalities (every entry of the N x N distance matrix is determined by the
class pair), so the whole computation reduces to:

  1. raw per-class segment sums R[c,d], T[c,d]  (memory-bound)
  2. the three 128x128 class Gram matrices P1 = R R^T, P2 = T T^T, P3 = R T^T
  3. tiny 128x128 class-pair loss math with the class counts

Device strategy (8 cores, feature/d-sharded so no cross-core collective is
needed): core k takes columns [256k, 256k+256) of both modal tensors and
computes the full-N segment sums for its d-chunk with one-hot matmuls on
the PE.  The data ships as fp8 e4m3 (1 byte/elem): the loss averages the
quantization noise over ~32 samples/class x 2048 dims, so the end-to-end
error stays ~6e-4 -- far inside the 2e-2 gate -- while HBM traffic drops
4x vs fp32 (2 MiB/core).  Both modals are interleaved per 128-sample block
([x1_b | x2_b] = 512 fp8 cols) and two sample blocks are contracted per
matmul with DoubleRow fp8 (2 elems/PE cell): 16 matmuls of 256-contraction
x 512-free, halving both the weight loads and the streaming cycles vs
single-pump.  Even pairs accumulate in PSUM bank a, odd pairs in bank b,
so consecutive matmuls pipeline; the DVE adds the banks during the fp16
output cast.  The one-hot operand for the first two chunks ships
precomputed in the first DMA so the PE starts without waiting for the
targets round-trip; the rest is generated on-device (gpsimd iota + one DVE
is_equal per chunk).  The host forms the three Grams and does the count
scaling + sqrt/relu/weighted mean (<0.1% of the FLOPs) in float64.
"""

import numpy as np
import ml_dtypes

import concourse.bacc as bacc
import concourse.bass as bass
import concourse.mybir as mybir
from concourse.bass_utils import run_bass_kernel_spmd

N = 4096
D = 2048
C = 128
MARGIN = 0.5
NCORES = 8
DCHUNK = D // NCORES          # 256 feature columns per core
P = 128                       # partitions / sample-block size
NB = N // P                   # 32 sample blocks
W = 2 * DCHUNK                # 512 interleaved fp8 cols per block (x1|x2)
# x-DMA chunking (in 128-sample blocks, all even so DoubleRow pairs never
# straddle a chunk).  Each dma_start costs ~0.7us of HWDGE issue (128
# descriptors) and descriptor size = blocks*512 B, so big middle chunks
# run near the 358 GB/s HBM limit while the small head/tail chunks
# shorten the first-data and last-chunk completion latencies.
CHUNKS = [2, 4, 8, 8, 8, 2]
NCHUNK = len(CHUNKS)
CHUNK_OFF = [sum(CHUNKS[:i]) for i in range(NCHUNK + 1)]  # block offsets
SYNC_CHUNKS = (0, 2, 4)       # chunk j on sync ring; others on scalar ring
OH0_BLOCKS = CHUNK_OFF[2]     # blocks with host-precomputed one-hot (chunks 0,1)

F32 = mybir.dt.float32
F16 = mybir.dt.float16
I32 = mybir.dt.int32
BF16 = mybir.dt.bfloat16
F8 = mybir.dt.float8e4
NPF8 = ml_dtypes.float8_e4m3

_PROGRAM = None


def _build_program() -> bass.Bass:
    """Raw-bass program (no TileContext): 5 engine streams.

    sync ring:   oh0 + x chunks 0,2,4 -> R-half output DMA
    scalar ring: targets + x chunks 1,3,5 -> T-half output DMA
    gpsimd:      iota row for the one-hot compare
    tensor:      HAM warmup, then one DoubleRow matmul per block pair
    vector:      per-chunk one-hot is_equal, final PSUM add + fp16 cast
    """
    nc = bass.Bass()

    # tgt[p, b] = targets[b*128 + p]; x[p, b, 0:256] = modal1 block b (this
    # core's d-chunk), x[p, b, 256:512] = modal2 block b -- packed host-side.
    # oh0 = precomputed one-hot for blocks [0, OH0_BLOCKS).
    tgt_in = nc.declare_dram_parameter("tgt", [P, NB], F32, isOutput=False)
    oh0_in = nc.declare_dram_parameter("oh0", [P, OH0_BLOCKS, C], F8, isOutput=False)
    x_in = nc.declare_dram_parameter("x", [P, NB, W], F8, isOutput=False)
    # sums[:, 0:256] = R segment sums, [:, 256:512] = T, fp16
    sums_out = nc.declare_dram_parameter("sums", [P, 512], F16, isOutput=True)

    import contextlib

    with contextlib.ExitStack() as stack:
        oh_t = stack.enter_context(nc.sbuf_tensor([P, NB, C], F8))
        tgt_t = stack.enter_context(nc.sbuf_tensor([P, NB], F32))
        iota_t = stack.enter_context(nc.sbuf_tensor([P, C], I32))
        x_t = stack.enter_context(nc.sbuf_tensor([P, NB, W], F8))
        warm_t = stack.enter_context(nc.sbuf_tensor([P, 136], BF16))
        out_t = stack.enter_context(nc.sbuf_tensor([P, 512], F16))
        sb_b = stack.enter_context(nc.sbuf_tensor([P, 512], F32))
        # Two full banks, even pairs -> a, odd pairs -> b: consecutive
        # matmuls then hit different PSUM banks and pipeline.  The DVE adds
        # the banks during the output cast.
        psum_a = stack.enter_context(nc.psum_tensor([P, 512], F32))
        psum_b = stack.enter_context(nc.psum_tensor([P, 512], F32))
        psum_warm = stack.enter_context(nc.psum_tensor([P, 8], F32))

        def sem(name):
            return stack.enter_context(nc.semaphore(name))

        tgt_sem = sem("tgt_dma")
        oh0_sem = sem("oh0_dma")
        iota_sem = sem("iota_gen")
        oh_gen = sem("oh_gen")
        x_sems = [sem(f"x_dma_{j}") for j in range(NCHUNK)]
        pe_done = sem("pe_done")
        vec_done = sem("vec_done")
        dma_out = sem("dma_out")

        # Raw-bass semaphores are NOT cleared by the framework preamble;
        # stale values from whatever ran on the core before would satisfy
        # our waits early.  Clear them, then fence with the NRT pseudo
        # barrier (safe while bass sems are still being cleared).
        all_sems = (
            [tgt_sem, oh0_sem, iota_sem, oh_gen]
            + x_sems
            + [pe_done, vec_done, dma_out]
        )
        nums = sorted(h.num for h in all_sems)
        assert nums == list(range(nums[0], nums[0] + len(nums))), nums
        sem_range = range(nums[0], nums[-1] + 1)
        nc.gpsimd.dma_reset(sem_range)
        nc.gpsimd.sem_clear(sem_range)
        # init the PE-warmup scratch (hidden under the gpsimd clears; also
        # keeps the simulator's uninitialized-read check happy)
        nc.vector.memset(warm_t[:], 0)
        nc._nrt_pseudo_barrier()

        # no_gpsimd_drain: skip the ~5us GpSimd DGE drain at block exit; the
        # block-exit engine drains + barrier fence everything that remains.
        with nc.Block(no_gpsimd_drain=True) as block:

            @block.gpsimd
            def _(gpsimd: bass.BassEngine):
                gpsimd.iota(
                    iota_t[:], [[1, C]], channel_multiplier=0
                ).then_inc(iota_sem, 1)

            @block.sync
            def _(sync: bass.BassEngine):
                sync.dma_start(out=oh_t[:, 0:OH0_BLOCKS, :], in_=oh0_in[:]).then_inc(
                    oh0_sem, 16
                )
                for j in SYNC_CHUNKS:
                    sl = slice(CHUNK_OFF[j], CHUNK_OFF[j + 1])
                    sync.dma_start(
                        out=x_t[:, sl, :], in_=x_in[:, sl, :]
                    ).then_inc(x_sems[j], 16)
                sync.wait_ge(vec_done, 1)
                # no wait on dma_out: the NEFF exit sequence (engine drains +
                # DGE queue drain) runs concurrently with the output write's
                # HBM receipt, hiding ~2us of completion latency.
                sync.dma_start(
                    out=sums_out[:, 0:256], in_=out_t[:, 0:256]
                ).then_inc(dma_out, 16)

            @block.scalar
            def _(scalar: bass.BassEngine):
                scalar.dma_start(out=tgt_t[:], in_=tgt_in[:]).then_inc(tgt_sem, 16)
                for j in range(NCHUNK):
                    if j in SYNC_CHUNKS:
                        continue
                    sl = slice(CHUNK_OFF[j], CHUNK_OFF[j + 1])
                    scalar.dma_start(
                        out=x_t[:, sl, :], in_=x_in[:, sl, :]
                    ).then_inc(x_sems[j], 16)
                scalar.wait_ge(vec_done, 1)
                scalar.dma_start(
                    out=sums_out[:, 256:512], in_=out_t[:, 256:512]
                ).then_inc(dma_out, 16)

            @block.tensor
            def _(tensor: bass.BassEngine):
                # Keep the PE HAM activity window busy (junk matmuls on
                # zeroed scratch) while the first DMA chunks land, so the
                # ~3.4us warmup clock runs from block entry instead of from
                # first data.
                for _ in range(12):
                    nc.tensor.matmul(
                        psum_warm[:],
                        warm_t[:, 0:128],
                        warm_t[:, 128:136],
                        start=True,
                        stop=True,
                    )
                for j in range(NCHUNK):
                    if j < 2:
                        tensor.wait_ge(oh0_sem, 16)
                    else:
                        tensor.wait_ge(oh_gen, j - 1)
                    tensor.wait_ge(x_sems[j], 16)
                    for pr in range(CHUNK_OFF[j] // 2, CHUNK_OFF[j + 1] // 2):
                        b = 2 * pr
                        nc.tensor.matmul(
                            (psum_a if pr % 2 == 0 else psum_b)[:, :],
                            oh_t[:, b : b + 2, :],
                            x_t[:, b : b + 2, :],
                            start=(pr < 2),
                            stop=(pr >= NB // 2 - 2),
                            perf_mode=mybir.MatmulPerfMode.DoubleRow,
                        )
                # drain makes sure the last matmul's PSUM writes have landed
                # before the DVE reads them.
                tensor.drain().then_inc(pe_done, 1)

            @block.vector
            def _(vector: bass.BassEngine):
                # One is_equal per x-chunk (from chunk 2 on; 0-1 ship
                # precomputed) builds that chunk's one-hot blocks:
                # oh[p, b, c] = (targets[b*128+p] == c), via broadcast APs
                # (iota row repeated per block, target column repeated per c).
                vector.wait_ge(tgt_sem, 16)
                vector.wait_ge(iota_sem, 1)
                for j in range(2, NCHUNK):
                    c0, c1 = CHUNK_OFF[j], CHUNK_OFF[j + 1]
                    nb = c1 - c0
                    iota_bc = bass.AP(iota_t, 0, [[C, P], [0, nb], [1, C]])
                    tgt_bc = bass.AP(tgt_t, c0, [[NB, P], [1, nb], [0, C]])
                    nc.vector.tensor_tensor(
                        oh_t[:, c0:c1, :],
                        iota_bc,
                        tgt_bc,
                        mybir.AluOpType.is_equal,
                    ).then_inc(oh_gen, 1)
                vector.wait_ge(pe_done, 1)
                # DVE may read only one PSUM operand per op, and same-engine
                # RAW needs an explicit drain (DVE execution pipelines), so
                # stage bank b through SBUF, drain, then add with fp16 cast.
                nc.vector.tensor_copy(sb_b[:, :], psum_b[:, :])
                vector.drain()
                nc.vector.tensor_tensor(
                    out_t[:, :], psum_a[:, :], sb_b[:, :], mybir.AluOpType.add
                )
                vector.drain().then_inc(vec_done, 1)

    return nc


def _get_program() -> bass.Bass:
    global _PROGRAM
    if _PROGRAM is None:
        _PROGRAM = _build_program()
    return _PROGRAM


def _make_in_maps(modal1, modal2, targets):
    x1 = np.asarray(modal1, dtype=np.float32).astype(NPF8)
    x2 = np.asarray(modal2, dtype=np.float32).astype(NPF8)
    targets = np.asarray(targets)

    tgt_pb = np.ascontiguousarray(
        targets.reshape(NB, P).T.astype(np.float32)
    )  # [p, b] = targets[b*128+p]

    # precomputed one-hot for blocks [0, OH0_BLOCKS)
    oh0 = (
        tgt_pb[:, :OH0_BLOCKS, None] == np.arange(C, dtype=np.float32)[None, None, :]
    ).astype(NPF8)
    oh0 = np.ascontiguousarray(oh0)

    # [4096, D] -> [128, NB, D'] with [p, b] = sample b*128+p
    x1b = x1.reshape(NB, P, D).transpose(1, 0, 2)
    x2b = x2.reshape(NB, P, D).transpose(1, 0, 2)

    in_maps = []
    for k in range(NCORES):
        sl = slice(k * DCHUNK, (k + 1) * DCHUNK)
        x = np.empty((P, NB, W), dtype=NPF8)
        x[:, :, :DCHUNK] = x1b[:, :, sl]
        x[:, :, DCHUNK:] = x2b[:, :, sl]
        in_maps.append({"tgt": tgt_pb, "oh0": oh0, "x": x})
    return in_maps


def _host_expected_sums(in_map):
    """Numpy model of the device output for one core (debug aid)."""
    tgt = in_map["tgt"].T.reshape(-1).astype(np.int64)  # sample order
    x = in_map["x"].astype(np.float32)  # [P, NB, W]
    flat = x.transpose(1, 0, 2).reshape(N, W)
    sums = np.zeros((C, W), np.float32)
    np.add.at(sums, tgt, flat)
    return sums.astype(np.float16)


def _finish_on_host(sums_list, targets):
    """Form class Grams from the per-core segment sums and do the loss."""
    P1 = np.zeros((C, C), np.float64)
    P2 = np.zeros((C, C), np.float64)
    P3 = np.zeros((C, C), np.float64)
    for s in sums_list:
        s = np.asarray(s, np.float64)
        R = s[:, 0:256]                      # [class, d-chunk]
        T = s[:, 256:512]
        P1 += R @ R.T
        P2 += T @ T.T
        P3 += R @ T.T

    n = np.bincount(targets, minlength=C).astype(np.float64)
    u = 1.0 / np.maximum(n, 1.0)

    S_CC = P1 + P2 + P3 + P3.T  # (R+T)(R+T)^T
    uu = np.outer(u, u)
    A1 = 0.5 * uu * (P1 + P3)    # meanR . ctr
    A2 = 0.5 * uu * (P2 + P3.T)  # meanT . ctr
    nR = u * u * np.diag(P1)
    nT = u * u * np.diag(P2)
    nCtr = 0.25 * u * u * np.diag(S_CC)

    Wgt = np.outer(n, n)
    eye = np.eye(C)
    total = 0.0
    for A, nrm in ((A1, nR), (A2, nT)):
        sq = np.maximum(nrm[:, None] + nCtr[None, :] - 2.0 * A, 1e-12)
        d = np.sqrt(sq)
        dd = np.sqrt(d + 1e-10)
        term = eye * sq + (1.0 - eye) * np.maximum(MARGIN - dd, 0.0) ** 2
        total += (Wgt * term).sum() / (float(N) * float(N))
    return np.asarray(total, dtype=np.float32)


def kernel(modal1_inputs, modal2_inputs, targets):
    nc = _get_program()
    in_maps = _make_in_maps(modal1_inputs, modal2_inputs, targets)
    res = run_bass_kernel_spmd(nc, in_maps, list(range(NCORES)))
    sums_list = [res.results[k]["sums"] for k in range(NCORES)]
    return _finish_on_host(sums_list, np.asarray(targets))
